# revision 11
# baseline (speedup 1.0000x reference)
"""Trainium2 Bass kernel for nn_BottleneckBit (ResNet bottleneck with ternary-
quantized convs + BN + SiLU + residual).

Strategy:
- Data-parallel over batch: 64 images -> 8 cores x 8 images.
- All convs lowered to TensorEngine matmuls with channels on partitions:
    conv1 (1x1, 1024->256):  8 K-tiles of 128, k-OUTER so the PE starts
                             streaming as soon as the first x chunk lands
    conv2 (3x3, 256->256):   9 shifted-tap matmuls x 2 K-tiles, reading from a
                             zero-padded 16x16-per-image SBUF buffer via 4D APs
    conv3 (1x1, 256->1024):  2 K-tiles; the residual x is added into PSUM by
                             the (otherwise idle) VectorEngine
- Ternary weight trick: wq = clip(round(w/s),-1,1)*s. The {-1,0,1} ternary part
  is exact in bf16; the per-out-channel scale s and the BN affine fold into a
  single per-channel (a, c): bn(conv(x, wq) + b) = a * conv_t(x) + c.
  a rides the ScalarEngine activation's per-partition scale operand (layers
  1/2) or is folded into the bf16 weights (layer 3, because the residual is
  already in PSUM pre-activation), c rides the bias operand. SiLU+affine is a
  single ACT instruction per tile.
- Activations in bf16 (products with ternary weights are exact in fp32 PSUM);
  output is stored bf16 on-device and upcast to fp32 on host.
"""
import numpy as np
import ml_dtypes

import concourse.bass as bass
import concourse.mybir as mybir
from concourse import bacc
from concourse.tile import TileContext
from concourse.bass_utils import run_bass_kernel_spmd

BN_EPS = 1e-5
Q_EPS = 1e-8

# Problem shape (hardcoded per contract)
B, CIN, H, W = 64, 1024, 14, 14
WIDTH, COUT = 256, 1024
N_CORES = 8
BC = B // N_CORES          # images per core = 8
PIX = H * W                # 196
P = 128
CIN_T = CIN // P           # 8
W_T = WIDTH // P           # 2
COUT_T = COUT // P         # 8
IPG = 2                    # images per matmul group
BP = BC // IPG             # 4 image-pair groups per core
NN = IPG * PIX             # 392 columns per matmul (<=512 PSUM bank)
HP, WP = H + 2, W + 2      # 16x16 padded image for the 3x3 conv
PADPIX = HP * WP           # 256
XCH = 2                    # x DMA granularity: k-tiles per chunk
JG = 4                     # layer-3 output-channel tiles batched per out DMA

_F32 = mybir.dt.float32
_BF16 = mybir.dt.bfloat16
_AF = mybir.ActivationFunctionType


def build(act_func=None):
    """Build the per-core Bass program (SPMD: same program on all 8 cores)."""
    if act_func is None:
        act_func = _AF.Silu
    nc = bacc.Bacc()

    xd = nc.declare_dram_parameter("x", [P, CIN_T * BC * PIX], _BF16, isOutput=False)
    w1d = nc.declare_dram_parameter("w1", [P, CIN_T * WIDTH], _BF16, isOutput=False)
    w2d = nc.declare_dram_parameter("w2", [P, W_T * 9 * WIDTH], _BF16, isOutput=False)
    w3d = nc.declare_dram_parameter("w3", [P, W_T * COUT], _BF16, isOutput=False)
    ccd = nc.declare_dram_parameter("cc", [P, 8 + COUT_T], _F32, isOutput=False)
    # output stays in the partition-folded layout [p, j*BC*PIX + n]; host unfolds
    outd = nc.declare_dram_parameter("out", [P, COUT_T * BC * PIX], _BF16, isOutput=True)

    with TileContext(nc) as tc:
        with tc.tile_pool(name="weights", bufs=1) as wpool, \
             tc.tile_pool(name="acts", bufs=1) as apool, \
             tc.tile_pool(name="outs", bufs=4) as opool, \
             tc.tile_pool(name="psum", bufs=8, space="PSUM") as pspool:

            # ---- DMA order = critical path order: first x chunk + w1 unblock
            # the PE; consts unblock the first ACT; w2/w3 are needed later ----
            w1t = wpool.tile([P, CIN_T * WIDTH], _BF16, name="w1t")
            nc.sync.dma_start(out=w1t[:, 0:WIDTH], in_=w1d[:, 0:WIDTH])
            xt = apool.tile([P, CIN_T * BC * PIX], _BF16, name="xt")
            nc.sync.dma_start(out=xt[:, 0:XCH * BC * PIX], in_=xd[:, 0:XCH * BC * PIX])
            cct = wpool.tile([P, 8 + COUT_T], _F32, name="cct")
            nc.sync.dma_start(out=cct[:, :], in_=ccd[:, :])
            for c in range(1, CIN_T // XCH):
                lo, hi = c * XCH * BC * PIX, (c + 1) * XCH * BC * PIX
                nc.sync.dma_start(out=xt[:, lo:hi], in_=xd[:, lo:hi])
            nc.sync.dma_start(out=w1t[:, WIDTH:], in_=w1d[:, WIDTH:])
            w2t = wpool.tile([P, W_T * 9 * WIDTH], _BF16, name="w2t")
            nc.sync.dma_start(out=w2t[:, :], in_=w2d[:, :])
            w3t = wpool.tile([P, W_T * COUT], _BF16, name="w3t")
            nc.sync.dma_start(out=w3t[:, :], in_=w3d[:, :])

            def xs(k, bp):          # x rhs slice [128, NN] for (ktile, bpair)
                return xt[:, k * BC * PIX + bp * NN: k * BC * PIX + (bp + 1) * NN]

            # ---- PE clock pre-warm: the HAM throttle needs ~3.4us of sustained
            # PE activity to lift the 1.2->2.4GHz clock gate. A few dummy
            # matmuls on a zeroed tile during the DMA lead-in start that window
            # early so the real stream runs warm almost immediately ----
            wsrc = apool.tile([P, 512], _BF16, name="wsrc")
            nc.vector.memset(wsrc[:, :], 0.0)
            wps = pspool.tile([P, NN], _F32, name="wps", tag="ps")
            for _ in range(24):
                nc.tensor.matmul(wps[:, 0:P], wsrc[:, 0:P], wsrc[:, 0:P],
                                 start=True, stop=True)

            # ---- padded h1 buffers (zero halo for the 3x3 conv) ----
            h1p = [[None] * BP for _ in range(W_T)]
            for j in range(W_T):
                for bp in range(BP):
                    t = apool.tile([P, IPG * PADPIX], _BF16, name=f"h1p{j}_{bp}")
                    nc.vector.memset(t[:, :], 0.0)
                    h1p[j][bp] = t
            h2 = [[None] * BP for _ in range(W_T)]
            for j in range(W_T):
                for bp in range(BP):
                    h2[j][bp] = apool.tile([P, NN], _BF16, name=f"h2{j}_{bp}")

            def l1_act(ps, j, bp):
                src = ps.rearrange("p (i r c) -> p i r c", i=IPG, r=H, c=W)
                dst = h1p[j][bp].rearrange(
                    "p (i r c) -> p i r c", i=IPG, r=HP, c=WP)[:, :, 1:1 + H, 1:1 + W]
                nc.scalar.activation(dst, src, act_func,
                                     bias=cct[:, 2 + j:3 + j], scale=cct[:, 0 + j:1 + j])

            # ---- layer 1: 1x1 conv 1024->256. k-OUTER over all 8 PSUM groups
            # so the PE streams while the rest of x arrives chunk by chunk ----
            def l1_all():
                ps1 = {}
                for bp in range(BP):
                    for j in range(W_T):
                        ps1[bp, j] = pspool.tile([P, NN], _F32,
                                                 name=f"ps1_{bp}_{j}", tag="ps")
                for k in range(CIN_T):
                    for bp in range(BP):
                        for j in range(W_T):
                            nc.tensor.matmul(
                                ps1[bp, j][:, :],
                                w1t[:, k * WIDTH + j * P: k * WIDTH + (j + 1) * P],
                                xs(k, bp),
                                start=(k == 0), stop=(k == CIN_T - 1))
                for bp in range(BP):
                    for j in range(W_T):
                        l1_act(ps1[bp, j], j, bp)

            # ---- layer 2: 3x3 conv 256->256 via 9 shifted taps ----
            def l2(bp):
                for j in range(W_T):
                    ps = pspool.tile([P, NN], _F32, name="ps2", tag="ps")
                    idx = 0
                    for tap in range(9):
                        dy, dx = tap // 3, tap % 3
                        for k in range(W_T):
                            rhs = h1p[k][bp].rearrange(
                                "p (i r c) -> p i r c", i=IPG, r=HP, c=WP
                            )[:, :, dy:dy + H, dx:dx + W]
                            nc.tensor.matmul(
                                ps[:, :],
                                w2t[:, k * 9 * WIDTH + tap * WIDTH + j * P:
                                    k * 9 * WIDTH + tap * WIDTH + (j + 1) * P],
                                rhs,
                                start=(idx == 0), stop=(idx == 9 * W_T - 1))
                            idx += 1
                    nc.scalar.activation(h2[j][bp][:, :], ps[:, :], act_func,
                                         bias=cct[:, 6 + j:7 + j],
                                         scale=cct[:, 4 + j:5 + j])

            # ---- layer 3: 1x1 conv 256->1024; residual added to PSUM by the
            # VectorEngine; ACT does silu(psum + c3) -> bf16; out DMAs batched
            # JG output-channel tiles at a time into the folded DRAM layout ----
            def l3(bp):
                for j0 in range(0, COUT_T, JG):
                    ot = opool.tile([P, JG * NN], _BF16, name="ot", tag="ot")
                    for dj in range(JG):
                        j = j0 + dj
                        ps = pspool.tile([P, NN], _F32, name="ps3", tag="ps")
                        for k in range(W_T):
                            nc.tensor.matmul(
                                ps[:, :],
                                w3t[:, k * COUT + j * P: k * COUT + (j + 1) * P],
                                h2[k][bp][:, :],
                                start=(k == 0), stop=(k == W_T - 1))
                        nc.vector.tensor_add(out=ps[:, :], in0=ps[:, :],
                                             in1=xs(j, bp))
                        nc.scalar.activation(
                            ot[:, dj * NN:(dj + 1) * NN], ps[:, :], act_func,
                            bias=cct[:, 8 + j:9 + j], scale=1.0)
                    # one DMA for JG j-tiles: DRAM view [P, JG, NN] strided
                    dst = outd.rearrange("p (j n) -> p j n", j=COUT_T)[
                        :, j0:j0 + JG, bp * NN:(bp + 1) * NN]
                    src = ot.rearrange("p (j n) -> p j n", j=JG)
                    nc.sync.dma_start(out=dst, in_=src)

            l1_all()
            l2(0)
            l2(1)
            l3(0)
            l2(2)
            l3(1)
            l2(3)
            l3(2)
            l3(3)

    nc.finalize()
    return nc


def _prep_host(x, w1, b1, g1, be1, m1, v1,
               w2, b2, g2, be2, m2, v2,
               w3, b3, g3, be3, m3, v3):
    """Quantize weights, fold BN, and lay out device arrays."""
    def quant(w):
        w = np.asarray(w, np.float32)
        s = np.median(np.abs(w).reshape(w.shape[0], -1), axis=1)
        s = np.maximum(s, np.float32(Q_EPS)).astype(np.float32)
        t = np.clip(np.round(w / s[:, None, None, None]), -1.0, 1.0).astype(np.float32)
        return t, s

    def fold(s, b, g, be, m, v):
        sc = np.asarray(g, np.float64) / np.sqrt(np.asarray(v, np.float64) + BN_EPS)
        a = (np.asarray(s, np.float64) * sc).astype(np.float32)
        c = (np.asarray(b, np.float64) * sc + np.asarray(be, np.float64)
             - np.asarray(m, np.float64) * sc).astype(np.float32)
        return a, c

    t1, s1 = quant(w1)
    t2, s2 = quant(w2)
    t3, s3 = quant(w3)
    a1, c1 = fold(s1, b1, g1, be1, m1, v1)
    a2, c2 = fold(s2, b2, g2, be2, m2, v2)
    a3, c3 = fold(s3, b3, g3, be3, m3, v3)

    bf = ml_dtypes.bfloat16

    def part_fold(m2d):
        # [K, M] -> [128, (K//128)*M]: row k*128+p lands at [p, k*M+m]
        kk, mm = m2d.shape
        return np.ascontiguousarray(
            m2d.reshape(kk // P, P, mm).transpose(1, 0, 2).reshape(P, -1))

    w1_dev = part_fold(t1[:, :, 0, 0].T).astype(bf)
    # lhsT2[kin, tap*WIDTH + m] = t2[m, kin, dy, dx]
    w2_dev = part_fold(
        t2.transpose(1, 2, 3, 0).reshape(WIDTH, 9 * WIDTH)).astype(bf)
    w3_dev = part_fold((t3[:, :, 0, 0] * a3[:, None]).T).astype(bf)

    cc = np.zeros((P, 8 + COUT_T), np.float32)
    cc[:, 0:2] = a1.reshape(W_T, P).T
    cc[:, 2:4] = c1.reshape(W_T, P).T
    cc[:, 4:6] = a2.reshape(W_T, P).T
    cc[:, 6:8] = c2.reshape(W_T, P).T
    cc[:, 8:] = c3.reshape(COUT_T, P).T

    const = {"w1": w1_dev, "w2": w2_dev, "w3": w3_dev,
             "cc": np.ascontiguousarray(cc)}

    x = np.asarray(x, np.float32)
    in_maps = []
    for c in range(N_CORES):
        xc = x[c * BC:(c + 1) * BC].reshape(BC, CIN, PIX)
        xc = xc.transpose(1, 0, 2).reshape(CIN, BC * PIX)
        in_maps.append({"x": part_fold(xc).astype(bf), **const})
    return in_maps


def _run(inputs, trace=False, act_func=None, **spmd_kwargs):
    nc = build(act_func)
    in_maps = _prep_host(**inputs)
    res = run_bass_kernel_spmd(nc, in_maps, list(range(N_CORES)),
                               trace=trace, **spmd_kwargs)
    outs = []
    for c in range(N_CORES):
        of = res.results[c]["out"].astype(np.float32)    # folded [P, COUT_T*BC*PIX]
        oc = of.reshape(P, COUT_T, BC * PIX).transpose(1, 0, 2).reshape(COUT, BC * PIX)
        oc = oc.reshape(COUT, BC, PIX).transpose(1, 0, 2).reshape(BC, COUT, H, W)
        outs.append(oc)
    full = np.concatenate(outs, axis=0).astype(np.float32)
    return full, res


def kernel(**inputs):
    out, _ = _run(inputs)
    return out


# revision 12
# speedup vs baseline: 1.0081x; 1.0081x over previous
"""Trainium2 Bass kernel for nn_BottleneckBit (ResNet bottleneck with ternary-
quantized convs + BN + SiLU + residual).

Strategy:
- Data-parallel over batch: 64 images -> 8 cores x 8 images.
- All convs lowered to TensorEngine matmuls with channels on partitions:
    conv1 (1x1, 1024->256):  8 K-tiles of 128, k-OUTER so the PE starts
                             streaming as soon as the first x chunk lands
    conv2 (3x3, 256->256):   9 shifted-tap matmuls x 2 K-tiles, reading from a
                             zero-padded 16x16-per-image SBUF buffer via 4D APs
    conv3 (1x1, 256->1024):  2 K-tiles; the residual x is added into PSUM by
                             the (otherwise idle) VectorEngine
- Ternary weight trick: wq = clip(round(w/s),-1,1)*s. The {-1,0,1} ternary part
  is exact in bf16; the per-out-channel scale s and the BN affine fold into a
  single per-channel (a, c): bn(conv(x, wq) + b) = a * conv_t(x) + c.
  a rides the ScalarEngine activation's per-partition scale operand (layers
  1/2) or is folded into the bf16 weights (layer 3, because the residual is
  already in PSUM pre-activation), c rides the bias operand. SiLU+affine is a
  single ACT instruction per tile.
- Activations in bf16 (products with ternary weights are exact in fp32 PSUM);
  output is stored bf16 on-device and upcast to fp32 on host.
"""
import numpy as np
import ml_dtypes

import concourse.bass as bass
import concourse.mybir as mybir
from concourse import bacc
from concourse.tile import TileContext
from concourse.bass_utils import run_bass_kernel_spmd

BN_EPS = 1e-5
Q_EPS = 1e-8

# Problem shape (hardcoded per contract)
B, CIN, H, W = 64, 1024, 14, 14
WIDTH, COUT = 256, 1024
N_CORES = 8
BC = B // N_CORES          # images per core = 8
PIX = H * W                # 196
P = 128
CIN_T = CIN // P           # 8
W_T = WIDTH // P           # 2
COUT_T = COUT // P         # 8
IPG = 2                    # images per matmul group
BP = BC // IPG             # 4 image-pair groups per core
NN = IPG * PIX             # 392 columns per matmul (<=512 PSUM bank)
HP, WP = H + 2, W + 2      # 16x16 padded image for the 3x3 conv
PADPIX = HP * WP           # 256
XCH = 1                    # x DMA granularity: k-tiles per chunk
JG = 4                     # layer-3 output-channel tiles batched per out DMA

_F32 = mybir.dt.float32
_BF16 = mybir.dt.bfloat16
_AF = mybir.ActivationFunctionType


def build(act_func=None):
    """Build the per-core Bass program (SPMD: same program on all 8 cores)."""
    if act_func is None:
        act_func = _AF.Silu
    nc = bacc.Bacc()

    xd = nc.declare_dram_parameter("x", [P, CIN_T * BC * PIX], _BF16, isOutput=False)
    w1d = nc.declare_dram_parameter("w1", [P, CIN_T * WIDTH], _BF16, isOutput=False)
    w2d = nc.declare_dram_parameter("w2", [P, W_T * 9 * WIDTH], _BF16, isOutput=False)
    w3d = nc.declare_dram_parameter("w3", [P, W_T * COUT], _BF16, isOutput=False)
    ccd = nc.declare_dram_parameter("cc", [P, 8 + COUT_T], _F32, isOutput=False)
    # output stays in the partition-folded layout [p, j*BC*PIX + n]; host unfolds
    outd = nc.declare_dram_parameter("out", [P, COUT_T * BC * PIX], _BF16, isOutput=True)

    with TileContext(nc) as tc:
        with tc.tile_pool(name="weights", bufs=1) as wpool, \
             tc.tile_pool(name="acts", bufs=1) as apool, \
             tc.tile_pool(name="outs", bufs=4) as opool, \
             tc.tile_pool(name="psum", bufs=8, space="PSUM") as pspool:

            # ---- DMA order = critical path order: first x chunk + w1 unblock
            # the PE; consts unblock the first ACT; w2/w3 are needed later ----
            w1t = wpool.tile([P, CIN_T * WIDTH], _BF16, name="w1t")
            nc.sync.dma_start(out=w1t[:, 0:WIDTH], in_=w1d[:, 0:WIDTH])
            xt = apool.tile([P, CIN_T * BC * PIX], _BF16, name="xt")
            nc.sync.dma_start(out=xt[:, 0:XCH * BC * PIX], in_=xd[:, 0:XCH * BC * PIX])
            cct = wpool.tile([P, 8 + COUT_T], _F32, name="cct")
            nc.sync.dma_start(out=cct[:, :], in_=ccd[:, :])
            for c in range(1, CIN_T // XCH):
                lo, hi = c * XCH * BC * PIX, (c + 1) * XCH * BC * PIX
                nc.sync.dma_start(out=xt[:, lo:hi], in_=xd[:, lo:hi])
            nc.sync.dma_start(out=w1t[:, WIDTH:], in_=w1d[:, WIDTH:])
            w2t = wpool.tile([P, W_T * 9 * WIDTH], _BF16, name="w2t")
            nc.sync.dma_start(out=w2t[:, :], in_=w2d[:, :])
            w3t = wpool.tile([P, W_T * COUT], _BF16, name="w3t")
            nc.sync.dma_start(out=w3t[:, :], in_=w3d[:, :])

            def xs(k, bp):          # x rhs slice [128, NN] for (ktile, bpair)
                return xt[:, k * BC * PIX + bp * NN: k * BC * PIX + (bp + 1) * NN]

            # ---- PE clock pre-warm: the HAM throttle needs ~3.4us of sustained
            # PE activity to lift the 1.2->2.4GHz clock gate. A few dummy
            # matmuls on a zeroed tile during the DMA lead-in start that window
            # early so the real stream runs warm almost immediately ----
            wsrc = apool.tile([P, 512], _BF16, name="wsrc")
            nc.vector.memset(wsrc[:, :], 0.0)
            wps = pspool.tile([P, NN], _F32, name="wps", tag="ps")
            for _ in range(16):
                nc.tensor.matmul(wps[:, 0:P], wsrc[:, 0:P], wsrc[:, 0:P],
                                 start=True, stop=True)

            # ---- padded h1 buffers (zero halo for the 3x3 conv) ----
            h1p = [[None] * BP for _ in range(W_T)]
            for j in range(W_T):
                for bp in range(BP):
                    t = apool.tile([P, IPG * PADPIX], _BF16, name=f"h1p{j}_{bp}")
                    nc.vector.memset(t[:, :], 0.0)
                    h1p[j][bp] = t
            h2 = [[None] * BP for _ in range(W_T)]
            for j in range(W_T):
                for bp in range(BP):
                    h2[j][bp] = apool.tile([P, NN], _BF16, name=f"h2{j}_{bp}")

            def l1_act(ps, j, bp):
                src = ps.rearrange("p (i r c) -> p i r c", i=IPG, r=H, c=W)
                dst = h1p[j][bp].rearrange(
                    "p (i r c) -> p i r c", i=IPG, r=HP, c=WP)[:, :, 1:1 + H, 1:1 + W]
                nc.scalar.activation(dst, src, act_func,
                                     bias=cct[:, 2 + j:3 + j], scale=cct[:, 0 + j:1 + j])

            # ---- layer 1: 1x1 conv 1024->256. k-OUTER over all 8 PSUM groups
            # so the PE streams while the rest of x arrives chunk by chunk ----
            def l1_all():
                ps1 = {}
                for bp in range(BP):
                    for j in range(W_T):
                        ps1[bp, j] = pspool.tile([P, NN], _F32,
                                                 name=f"ps1_{bp}_{j}", tag="ps")
                for k in range(CIN_T):
                    for bp in range(BP):
                        for j in range(W_T):
                            nc.tensor.matmul(
                                ps1[bp, j][:, :],
                                w1t[:, k * WIDTH + j * P: k * WIDTH + (j + 1) * P],
                                xs(k, bp),
                                start=(k == 0), stop=(k == CIN_T - 1))
                for bp in range(BP):
                    for j in range(W_T):
                        l1_act(ps1[bp, j], j, bp)

            # ---- layer 2: 3x3 conv 256->256 via 9 shifted taps ----
            def l2(bp):
                for j in range(W_T):
                    ps = pspool.tile([P, NN], _F32, name="ps2", tag="ps")
                    idx = 0
                    for tap in range(9):
                        dy, dx = tap // 3, tap % 3
                        for k in range(W_T):
                            rhs = h1p[k][bp].rearrange(
                                "p (i r c) -> p i r c", i=IPG, r=HP, c=WP
                            )[:, :, dy:dy + H, dx:dx + W]
                            nc.tensor.matmul(
                                ps[:, :],
                                w2t[:, k * 9 * WIDTH + tap * WIDTH + j * P:
                                    k * 9 * WIDTH + tap * WIDTH + (j + 1) * P],
                                rhs,
                                start=(idx == 0), stop=(idx == 9 * W_T - 1))
                            idx += 1
                    nc.scalar.activation(h2[j][bp][:, :], ps[:, :], act_func,
                                         bias=cct[:, 6 + j:7 + j],
                                         scale=cct[:, 4 + j:5 + j])

            # ---- layer 3: 1x1 conv 256->1024; residual added to PSUM by the
            # VectorEngine; ACT does silu(psum + c3) -> bf16; out DMAs batched
            # JG output-channel tiles at a time into the folded DRAM layout ----
            def l3(bp):
                for j0 in range(0, COUT_T, JG):
                    ot = opool.tile([P, JG * NN], _BF16, name="ot", tag="ot")
                    for dj in range(JG):
                        j = j0 + dj
                        ps = pspool.tile([P, NN], _F32, name="ps3", tag="ps")
                        for k in range(W_T):
                            nc.tensor.matmul(
                                ps[:, :],
                                w3t[:, k * COUT + j * P: k * COUT + (j + 1) * P],
                                h2[k][bp][:, :],
                                start=(k == 0), stop=(k == W_T - 1))
                        nc.vector.tensor_add(out=ps[:, :], in0=ps[:, :],
                                             in1=xs(j, bp))
                        nc.scalar.activation(
                            ot[:, dj * NN:(dj + 1) * NN], ps[:, :], act_func,
                            bias=cct[:, 8 + j:9 + j], scale=1.0)
                    # one DMA for JG j-tiles: DRAM view [P, JG, NN] strided
                    dst = outd.rearrange("p (j n) -> p j n", j=COUT_T)[
                        :, j0:j0 + JG, bp * NN:(bp + 1) * NN]
                    src = ot.rearrange("p (j n) -> p j n", j=JG)
                    nc.sync.dma_start(out=dst, in_=src)

            l1_all()
            l2(0)
            l2(1)
            l3(0)
            l2(2)
            l3(1)
            l2(3)
            l3(2)
            l3(3)

    nc.finalize()
    return nc


def _prep_host(x, w1, b1, g1, be1, m1, v1,
               w2, b2, g2, be2, m2, v2,
               w3, b3, g3, be3, m3, v3):
    """Quantize weights, fold BN, and lay out device arrays."""
    def quant(w):
        w = np.asarray(w, np.float32)
        s = np.median(np.abs(w).reshape(w.shape[0], -1), axis=1)
        s = np.maximum(s, np.float32(Q_EPS)).astype(np.float32)
        t = np.clip(np.round(w / s[:, None, None, None]), -1.0, 1.0).astype(np.float32)
        return t, s

    def fold(s, b, g, be, m, v):
        sc = np.asarray(g, np.float64) / np.sqrt(np.asarray(v, np.float64) + BN_EPS)
        a = (np.asarray(s, np.float64) * sc).astype(np.float32)
        c = (np.asarray(b, np.float64) * sc + np.asarray(be, np.float64)
             - np.asarray(m, np.float64) * sc).astype(np.float32)
        return a, c

    t1, s1 = quant(w1)
    t2, s2 = quant(w2)
    t3, s3 = quant(w3)
    a1, c1 = fold(s1, b1, g1, be1, m1, v1)
    a2, c2 = fold(s2, b2, g2, be2, m2, v2)
    a3, c3 = fold(s3, b3, g3, be3, m3, v3)

    bf = ml_dtypes.bfloat16

    def part_fold(m2d):
        # [K, M] -> [128, (K//128)*M]: row k*128+p lands at [p, k*M+m]
        kk, mm = m2d.shape
        return np.ascontiguousarray(
            m2d.reshape(kk // P, P, mm).transpose(1, 0, 2).reshape(P, -1))

    w1_dev = part_fold(t1[:, :, 0, 0].T).astype(bf)
    # lhsT2[kin, tap*WIDTH + m] = t2[m, kin, dy, dx]
    w2_dev = part_fold(
        t2.transpose(1, 2, 3, 0).reshape(WIDTH, 9 * WIDTH)).astype(bf)
    w3_dev = part_fold((t3[:, :, 0, 0] * a3[:, None]).T).astype(bf)

    cc = np.zeros((P, 8 + COUT_T), np.float32)
    cc[:, 0:2] = a1.reshape(W_T, P).T
    cc[:, 2:4] = c1.reshape(W_T, P).T
    cc[:, 4:6] = a2.reshape(W_T, P).T
    cc[:, 6:8] = c2.reshape(W_T, P).T
    cc[:, 8:] = c3.reshape(COUT_T, P).T

    const = {"w1": w1_dev, "w2": w2_dev, "w3": w3_dev,
             "cc": np.ascontiguousarray(cc)}

    x = np.asarray(x, np.float32)
    in_maps = []
    for c in range(N_CORES):
        xc = x[c * BC:(c + 1) * BC].reshape(BC, CIN, PIX)
        xc = xc.transpose(1, 0, 2).reshape(CIN, BC * PIX)
        in_maps.append({"x": part_fold(xc).astype(bf), **const})
    return in_maps


def _run(inputs, trace=False, act_func=None, **spmd_kwargs):
    nc = build(act_func)
    in_maps = _prep_host(**inputs)
    res = run_bass_kernel_spmd(nc, in_maps, list(range(N_CORES)),
                               trace=trace, **spmd_kwargs)
    outs = []
    for c in range(N_CORES):
        of = res.results[c]["out"].astype(np.float32)    # folded [P, COUT_T*BC*PIX]
        oc = of.reshape(P, COUT_T, BC * PIX).transpose(1, 0, 2).reshape(COUT, BC * PIX)
        oc = oc.reshape(COUT, BC, PIX).transpose(1, 0, 2).reshape(BC, COUT, H, W)
        outs.append(oc)
    full = np.concatenate(outs, axis=0).astype(np.float32)
    return full, res


def kernel(**inputs):
    out, _ = _run(inputs)
    return out


# revision 13
# speedup vs baseline: 1.1200x; 1.1109x over previous
"""Trainium2 Bass kernel for nn_BottleneckBit (ResNet bottleneck with ternary-
quantized convs + BN + SiLU + residual).

Strategy:
- Data-parallel over batch: 64 images -> 8 cores x 8 images.
- All convs lowered to TensorEngine matmuls with channels on partitions:
    conv1 (1x1, 1024->256):  8 K-tiles of 128, k-OUTER so the PE starts
                             streaming as soon as the first x chunk lands
    conv2 (3x3, 256->256):   9 shifted-tap matmuls x 2 K-tiles, reading from a
                             zero-padded 16x16-per-image SBUF buffer via 4D APs
    conv3 (1x1, 256->1024):  2 K-tiles; the residual x is added into PSUM by
                             the (otherwise idle) VectorEngine
- Ternary weight trick: wq = clip(round(w/s),-1,1)*s. The {-1,0,1} ternary part
  is exact in bf16; the per-out-channel scale s and the BN affine fold into a
  single per-channel (a, c): bn(conv(x, wq) + b) = a * conv_t(x) + c.
  a rides the ScalarEngine activation's per-partition scale operand (layers
  1/2) or is folded into the bf16 weights (layer 3, because the residual is
  already in PSUM pre-activation), c rides the bias operand. SiLU+affine is a
  single ACT instruction per tile.
- Activations in bf16 (products with ternary weights are exact in fp32 PSUM);
  output is stored bf16 on-device and upcast to fp32 on host.
"""
import numpy as np
import ml_dtypes

import concourse.bass as bass
import concourse.mybir as mybir
from concourse import bacc
from concourse.tile import TileContext
from concourse.bass_utils import run_bass_kernel_spmd

BN_EPS = 1e-5
Q_EPS = 1e-8

# Problem shape (hardcoded per contract)
B, CIN, H, W = 64, 1024, 14, 14
WIDTH, COUT = 256, 1024
N_CORES = 8
BC = B // N_CORES          # images per core = 8
PIX = H * W                # 196
P = 128
CIN_T = CIN // P           # 8
W_T = WIDTH // P           # 2
COUT_T = COUT // P         # 8
IPG = 2                    # images per matmul group
BP = BC // IPG             # 4 image-pair groups per core
NN = IPG * PIX             # 392 columns per matmul (<=512 PSUM bank)
HP, WP = H + 2, W + 2      # 16x16 padded image for the 3x3 conv
PADPIX = HP * WP           # 256
XCH = 1                    # x DMA granularity: k-tiles per chunk
JG = 4                     # layer-3 output-channel tiles batched per out DMA

_F32 = mybir.dt.float32
_BF16 = mybir.dt.bfloat16
_AF = mybir.ActivationFunctionType


def build(act_func=None):
    """Build the per-core Bass program (SPMD: same program on all 8 cores)."""
    if act_func is None:
        act_func = _AF.Silu
    nc = bacc.Bacc()

    xd = nc.declare_dram_parameter("x", [P, CIN_T * BC * PIX], _BF16, isOutput=False)
    w1d = nc.declare_dram_parameter("w1", [P, CIN_T * WIDTH], _BF16, isOutput=False)
    w2d = nc.declare_dram_parameter("w2", [P, W_T * 9 * WIDTH], _BF16, isOutput=False)
    w3d = nc.declare_dram_parameter("w3", [P, W_T * COUT], _BF16, isOutput=False)
    ccd = nc.declare_dram_parameter("cc", [P, 8 + COUT_T], _F32, isOutput=False)
    # output stays in the partition-folded layout [p, j*BC*PIX + n]; host unfolds
    outd = nc.declare_dram_parameter("out", [P, COUT_T * BC * PIX], _BF16, isOutput=True)

    with TileContext(nc) as tc:
        with tc.tile_pool(name="weights", bufs=1) as wpool, \
             tc.tile_pool(name="acts", bufs=1) as apool, \
             tc.tile_pool(name="outs", bufs=4) as opool, \
             tc.tile_pool(name="psum", bufs=8, space="PSUM") as pspool:

            # ---- DMA order = critical path order: first x chunk + w1 unblock
            # the PE; consts unblock the first ACT; w2/w3 are needed later ----
            w1t = wpool.tile([P, CIN_T * WIDTH], _BF16, name="w1t")
            nc.sync.dma_start(out=w1t[:, 0:WIDTH], in_=w1d[:, 0:WIDTH])
            xt = apool.tile([P, CIN_T * BC * PIX], _BF16, name="xt")
            nc.sync.dma_start(out=xt[:, 0:XCH * BC * PIX], in_=xd[:, 0:XCH * BC * PIX])
            nc.sync.dma_start(out=w1t[:, WIDTH:], in_=w1d[:, WIDTH:])
            cct = wpool.tile([P, 8 + COUT_T], _F32, name="cct")
            nc.sync.dma_start(out=cct[:, :], in_=ccd[:, :])
            for c in range(1, CIN_T // XCH):
                lo, hi = c * XCH * BC * PIX, (c + 1) * XCH * BC * PIX
                nc.sync.dma_start(out=xt[:, lo:hi], in_=xd[:, lo:hi])
            w2t = wpool.tile([P, W_T * 9 * WIDTH], _BF16, name="w2t")
            nc.sync.dma_start(out=w2t[:, :], in_=w2d[:, :])
            w3t = wpool.tile([P, W_T * COUT], _BF16, name="w3t")
            nc.sync.dma_start(out=w3t[:, :], in_=w3d[:, :])

            def xs(k, bp):          # x rhs slice [128, NN] for (ktile, bpair)
                return xt[:, k * BC * PIX + bp * NN: k * BC * PIX + (bp + 1) * NN]

            # ---- PE clock pre-warm: the HAM throttle needs ~3.4us of sustained
            # PE activity to lift the 1.2->2.4GHz clock gate. A few dummy
            # matmuls on a zeroed tile during the DMA lead-in start that window
            # early so the real stream runs warm almost immediately ----
            wsrc = apool.tile([P, 512], _BF16, name="wsrc")
            nc.vector.memset(wsrc[:, :], 0.0)
            wps = pspool.tile([P, NN], _F32, name="wps", tag="ps")
            for _ in range(16):
                nc.tensor.matmul(wps[:, 0:P], wsrc[:, 0:P], wsrc[:, 0:P],
                                 start=True, stop=True)

            # ---- padded h1 buffers (zero halo for the 3x3 conv) ----
            h1p = [[None] * BP for _ in range(W_T)]
            for j in range(W_T):
                for bp in range(BP):
                    t = apool.tile([P, IPG * PADPIX], _BF16, name=f"h1p{j}_{bp}")
                    nc.vector.memset(t[:, :], 0.0)
                    h1p[j][bp] = t
            h2 = [[None] * BP for _ in range(W_T)]
            for j in range(W_T):
                for bp in range(BP):
                    h2[j][bp] = apool.tile([P, NN], _BF16, name=f"h2{j}_{bp}")

            def l1_act(ps, j, bp):
                src = ps.rearrange("p (i r c) -> p i r c", i=IPG, r=H, c=W)
                dst = h1p[j][bp].rearrange(
                    "p (i r c) -> p i r c", i=IPG, r=HP, c=WP)[:, :, 1:1 + H, 1:1 + W]
                nc.scalar.activation(dst, src, act_func,
                                     bias=cct[:, 2 + j:3 + j], scale=cct[:, 0 + j:1 + j])

            # ---- layer 1: 1x1 conv 1024->256. k-OUTER over all 8 PSUM groups
            # so the PE streams while the rest of x arrives chunk by chunk ----
            def l1_all():
                ps1 = {}
                for bp in range(BP):
                    for j in range(W_T):
                        ps1[bp, j] = pspool.tile([P, NN], _F32,
                                                 name=f"ps1_{bp}_{j}", tag="ps")
                for k in range(CIN_T):
                    for bp in range(BP):
                        for j in range(W_T):
                            nc.tensor.matmul(
                                ps1[bp, j][:, :],
                                w1t[:, k * WIDTH + j * P: k * WIDTH + (j + 1) * P],
                                xs(k, bp),
                                start=(k == 0), stop=(k == CIN_T - 1))
                for bp in range(BP):
                    for j in range(W_T):
                        l1_act(ps1[bp, j], j, bp)

            # ---- layer 2: 3x3 conv 256->256 via 9 shifted taps ----
            def l2(bp):
                for j in range(W_T):
                    ps = pspool.tile([P, NN], _F32, name="ps2", tag="ps")
                    idx = 0
                    for tap in range(9):
                        dy, dx = tap // 3, tap % 3
                        for k in range(W_T):
                            rhs = h1p[k][bp].rearrange(
                                "p (i r c) -> p i r c", i=IPG, r=HP, c=WP
                            )[:, :, dy:dy + H, dx:dx + W]
                            nc.tensor.matmul(
                                ps[:, :],
                                w2t[:, k * 9 * WIDTH + tap * WIDTH + j * P:
                                    k * 9 * WIDTH + tap * WIDTH + (j + 1) * P],
                                rhs,
                                start=(idx == 0), stop=(idx == 9 * W_T - 1))
                            idx += 1
                    nc.scalar.activation(h2[j][bp][:, :], ps[:, :], act_func,
                                         bias=cct[:, 6 + j:7 + j],
                                         scale=cct[:, 4 + j:5 + j])

            # ---- layer 3: 1x1 conv 256->1024; residual added to PSUM by the
            # VectorEngine; ACT does silu(psum + c3) -> bf16; out DMAs batched
            # JG output-channel tiles at a time into the folded DRAM layout ----
            def l3(bp):
                for j0 in range(0, COUT_T, JG):
                    ot = opool.tile([P, JG * NN], _BF16, name="ot", tag="ot")
                    for dj in range(JG):
                        j = j0 + dj
                        ps = pspool.tile([P, NN], _F32, name="ps3", tag="ps")
                        for k in range(W_T):
                            nc.tensor.matmul(
                                ps[:, :],
                                w3t[:, k * COUT + j * P: k * COUT + (j + 1) * P],
                                h2[k][bp][:, :],
                                start=(k == 0), stop=(k == W_T - 1))
                        nc.vector.tensor_add(out=ps[:, :], in0=ps[:, :],
                                             in1=xs(j, bp))
                        nc.scalar.activation(
                            ot[:, dj * NN:(dj + 1) * NN], ps[:, :], act_func,
                            bias=cct[:, 8 + j:9 + j], scale=1.0)
                    # one DMA for JG j-tiles: DRAM view [P, JG, NN] strided
                    dst = outd.rearrange("p (j n) -> p j n", j=COUT_T)[
                        :, j0:j0 + JG, bp * NN:(bp + 1) * NN]
                    src = ot.rearrange("p (j n) -> p j n", j=JG)
                    nc.sync.dma_start(out=dst, in_=src)

            l1_all()
            l2(0)
            l2(1)
            l3(0)
            l2(2)
            l3(1)
            l2(3)
            l3(2)
            l3(3)

    nc.finalize()
    return nc


def _prep_host(x, w1, b1, g1, be1, m1, v1,
               w2, b2, g2, be2, m2, v2,
               w3, b3, g3, be3, m3, v3):
    """Quantize weights, fold BN, and lay out device arrays."""
    def quant(w):
        w = np.asarray(w, np.float32)
        s = np.median(np.abs(w).reshape(w.shape[0], -1), axis=1)
        s = np.maximum(s, np.float32(Q_EPS)).astype(np.float32)
        t = np.clip(np.round(w / s[:, None, None, None]), -1.0, 1.0).astype(np.float32)
        return t, s

    def fold(s, b, g, be, m, v):
        sc = np.asarray(g, np.float64) / np.sqrt(np.asarray(v, np.float64) + BN_EPS)
        a = (np.asarray(s, np.float64) * sc).astype(np.float32)
        c = (np.asarray(b, np.float64) * sc + np.asarray(be, np.float64)
             - np.asarray(m, np.float64) * sc).astype(np.float32)
        return a, c

    t1, s1 = quant(w1)
    t2, s2 = quant(w2)
    t3, s3 = quant(w3)
    a1, c1 = fold(s1, b1, g1, be1, m1, v1)
    a2, c2 = fold(s2, b2, g2, be2, m2, v2)
    a3, c3 = fold(s3, b3, g3, be3, m3, v3)

    bf = ml_dtypes.bfloat16

    def part_fold(m2d):
        # [K, M] -> [128, (K//128)*M]: row k*128+p lands at [p, k*M+m]
        kk, mm = m2d.shape
        return np.ascontiguousarray(
            m2d.reshape(kk // P, P, mm).transpose(1, 0, 2).reshape(P, -1))

    w1_dev = part_fold(t1[:, :, 0, 0].T).astype(bf)
    # lhsT2[kin, tap*WIDTH + m] = t2[m, kin, dy, dx]
    w2_dev = part_fold(
        t2.transpose(1, 2, 3, 0).reshape(WIDTH, 9 * WIDTH)).astype(bf)
    w3_dev = part_fold((t3[:, :, 0, 0] * a3[:, None]).T).astype(bf)

    cc = np.zeros((P, 8 + COUT_T), np.float32)
    cc[:, 0:2] = a1.reshape(W_T, P).T
    cc[:, 2:4] = c1.reshape(W_T, P).T
    cc[:, 4:6] = a2.reshape(W_T, P).T
    cc[:, 6:8] = c2.reshape(W_T, P).T
    cc[:, 8:] = c3.reshape(COUT_T, P).T

    const = {"w1": w1_dev, "w2": w2_dev, "w3": w3_dev,
             "cc": np.ascontiguousarray(cc)}

    x = np.asarray(x, np.float32)
    in_maps = []
    for c in range(N_CORES):
        xc = x[c * BC:(c + 1) * BC].reshape(BC, CIN, PIX)
        xc = xc.transpose(1, 0, 2).reshape(CIN, BC * PIX)
        in_maps.append({"x": part_fold(xc).astype(bf), **const})
    return in_maps


def _run(inputs, trace=False, act_func=None, **spmd_kwargs):
    nc = build(act_func)
    in_maps = _prep_host(**inputs)
    res = run_bass_kernel_spmd(nc, in_maps, list(range(N_CORES)),
                               trace=trace, **spmd_kwargs)
    outs = []
    for c in range(N_CORES):
        of = res.results[c]["out"].astype(np.float32)    # folded [P, COUT_T*BC*PIX]
        oc = of.reshape(P, COUT_T, BC * PIX).transpose(1, 0, 2).reshape(COUT, BC * PIX)
        oc = oc.reshape(COUT, BC, PIX).transpose(1, 0, 2).reshape(BC, COUT, H, W)
        outs.append(oc)
    full = np.concatenate(outs, axis=0).astype(np.float32)
    return full, res


def kernel(**inputs):
    out, _ = _run(inputs)
    return out
